# revision 9
# baseline (speedup 1.0000x reference)
"""MobiusFlow Trainium2 kernel: 8-core data-parallel Bass/Tile implementation.

Math (validated against the reference to fp32 precision in debug_math.py):
  x = rot[:,:,0], y = rot[:,:,1]   (NOT unit vectors)
  cond = [feature, y, vec[:, :3]] @ W1p -> relu -> @ W2p  (f32r matmuls)
  conds = [weights(64) | w_raw c-major(192) | real_b(3)]  (real_s dropped: exp*0+1)
  dyw,dxt,dvw,q = per-k dots of w_raw with y, x~=x-(x.y)y, v=norm(cross(x,y)), self
  g2 = max(q-(2-|y|^2)dyw^2, 0); rc = 1/(1+sqrt(g2)); c = 0.7*rc; wn2 = c^2 g2
  u_x = c*dxt; u_v = c*dvw; zwn2 = |x|^2 - 2u_x + wn2; a = (1-wn2)/zwn2
  hv2 = (1+a)|x| u_v  (= -hv_true*|x|);  hr = (1+a)u_x - a|x|^2  (= hr_true*|x|)
  rad = wrapped atan2(hv_true, hr) via half-angle q = hv2/(|(hv2,hr)| + |hr|):
        E' = -2*atan(q) [deg-7 poly]; rad = E' - 2*mx*E' + pi*mx + 2pi*(1-mx)*my
        with mx = [hr<0], my = [hv2>0]
  wsp = softplus = Ln(Exp(raw)+1); t = sum(wsp*rad)/sum(wsp)
  dtx = sum(wsp*a)/sum(wsp)  (Householder norm-preservation; |dz_dtheta|=1)
  tx = -(x/|x|) cos t + v sin t;  tz = norm(cross(tx, y));  ldj = Ln(dtx)
  tvec = [vec[:, :3], vec[:, 3:] + real_b]
"""
import numpy as np

import concourse.bass as bass
import concourse.bacc as bacc
import concourse.mybir as mybir
import concourse.tile as tile
from concourse import masks
from concourse.bass_utils import run_bass_kernel_spmd

F32 = mybir.dt.float32
F32R = mybir.dt.float32r
AF = mybir.ActivationFunctionType
ALU = mybir.AluOpType
AX = mybir.AxisListType

NCORES = 8
N_TOTAL = 131072
R = N_TOTAL // NCORES          # rows per core
T = 16                         # row-blocks per supertile
ST_ROWS = 128 * T              # 2048
NST = R // ST_ROWS             # 8 supertiles
NG = ST_ROWS // 512            # 4 groups of 512 rows per supertile

K = 64
PI = float(np.pi)
TWO_PI = 2.0 * PI

# atan deg-7 odd minimax on [-1,1] (fit_polys.py)
AT_C0, AT_C1, AT_C2, AT_C3 = (0.9992138034627527, -0.32117489148664036,
                              0.14626430128714862, -0.03898641853047838)
# sin deg-11 odd on [-pi, pi]
SIN_C = (0.999999603917376, -0.16666553446863705, 0.008332407595463078,
         -0.00019808740070592645, 2.6998228073890683e-06,
         -2.0366231290827938e-08)

_BUILD_CACHE = {}


def build_bass(nst=NST):
    nc = bacc.Bacc("TRN2", target_bir_lowering=False)

    rot = nc.dram_tensor("rot", [R, 9], F32, kind="ExternalInput")
    vec = nc.dram_tensor("vec", [R, 6], F32, kind="ExternalInput")
    feat = nc.dram_tensor("feat", [R, 256], F32, kind="ExternalInput")
    W1p = nc.dram_tensor("W1p", [262, 256], F32, kind="ExternalInput")
    b1d = nc.dram_tensor("b1d", [256], F32, kind="ExternalInput")
    W2p = nc.dram_tensor("W2p", [256, 259], F32, kind="ExternalInput")
    b2d = nc.dram_tensor("b2d", [259], F32, kind="ExternalInput")

    trot_o = nc.dram_tensor("trot_o", [R, 9], F32, kind="ExternalOutput")
    tvec_o = nc.dram_tensor("tvec_o", [R, 6], F32, kind="ExternalOutput")
    ldj_o = nc.dram_tensor("ldj_o", [R], F32, kind="ExternalOutput")

    SH = [128, T, K]
    S1 = [128, T]
    S3 = [128, T, 3]

    def b1c(ap_small):          # [128,T] -> [128,T,1] view
        return ap_small.unsqueeze(2)

    with tile.TileContext(nc) as tc:
        with tc.tile_pool(name="wpool", bufs=1) as wp, \
             tc.tile_pool(name="io", bufs=2) as io, \
             tc.tile_pool(name="mlp", bufs=2) as mp, \
             tc.tile_pool(name="csp", bufs=2) as csp, \
             tc.tile_pool(name="bb", bufs=2) as bb, \
             tc.tile_pool(name="sm", bufs=2) as sm, \
             tc.tile_pool(name="outp", bufs=2) as op_, \
             tc.tile_pool(name="ps_t", bufs=1, space="PSUM") as ps_t, \
             tc.tile_pool(name="ps_mm", bufs=1, space="PSUM") as ps_mm, \
             tc.tile_pool(name="ps_bk", bufs=2, space="PSUM") as ps_bk:

            # ---------------- weights / constants (once) ----------------
            ident = wp.tile([128, 128], F32)
            masks.make_identity(nc, ident[:])

            w1_f32 = [wp.tile([128, 256], F32, name="w1f_0"),
                      wp.tile([128, 256], F32, name="w1f_1"),
                      wp.tile([6, 256], F32, name="w1f_2")]
            nc.sync.dma_start(w1_f32[0][:], W1p[0:128, :])
            nc.sync.dma_start(w1_f32[1][:], W1p[128:256, :])
            nc.sync.dma_start(w1_f32[2][:], W1p[256:262, :])
            w1 = [wp.tile([128, 256], F32R, name="w1r_0"),
                  wp.tile([128, 256], F32R, name="w1r_1"),
                  wp.tile([6, 256], F32R, name="w1r_2")]
            for a, b in zip(w1_f32, w1):
                nc.vector.tensor_copy(b[:], a[:])

            w2_f32 = [wp.tile([128, 259], F32, name="w2f_0"),
                      wp.tile([128, 259], F32, name="w2f_1")]
            nc.sync.dma_start(w2_f32[0][:], W2p[0:128, :])
            nc.sync.dma_start(w2_f32[1][:], W2p[128:256, :])
            w2 = [wp.tile([128, 259], F32R, name="w2r_0"),
                  wp.tile([128, 259], F32R, name="w2r_1")]
            for a, b in zip(w2_f32, w2):
                nc.vector.tensor_copy(b[:], a[:])

            b1t = wp.tile([128, 2], F32)
            nc.sync.dma_start(b1t[:], b1d[:].rearrange("(j p) -> p j", p=128))
            b2t = wp.tile([128, 3], F32)
            nc.sync.dma_start(b2t[:, 0:2],
                              b2d[0:256].rearrange("(j p) -> p j", p=128))
            nc.sync.dma_start(b2t[0:3, 2:3], b2d[256:259].unsqueeze(1))

            for st in range(nst):
                base = st * ST_ROWS

                rot_t = io.tile([128, T, 9], F32, name="rot_t")
                nc.sync.dma_start(
                    rot_t[:], rot[base:base + ST_ROWS, :]
                    .rearrange("(p t) c -> p t c", t=T))
                vec_t = io.tile([128, T, 6], F32, name="vec_t")
                nc.sync.dma_start(
                    vec_t[:], vec[base:base + ST_ROWS, :]
                    .rearrange("(p t) c -> p t c", t=T))

                cs = csp.tile([128, T, 259], F32, name="cs")

                # ---------------- MLP per 512-row group ----------------
                for g in range(NG):
                    feat_g = io.tile([128, 4, 256], F32, name="feat_g")
                    nc.sync.dma_start(
                        feat_g[:],
                        bass.AP(tensor=feat, offset=(base + g * 4) * 256,
                                ap=[[256 * T, 128], [256, 4], [1, 256]]))
                    yv6n = mp.tile([128, 4, 6], F32, name="yv6n")
                    nc.gpsimd.tensor_copy(yv6n[:, :, 0:3],
                                          rot_t[:, g * 4:g * 4 + 4, 1:9:3])
                    nc.gpsimd.tensor_copy(yv6n[:, :, 3:6],
                                          vec_t[:, g * 4:g * 4 + 4, 0:3])

                    pci0 = ps_t.tile([128, 512], F32, name="pci0")
                    pci1 = ps_t.tile([128, 512], F32, name="pci1")
                    pci2 = ps_t.tile([6, 512], F32, name="pci2")
                    for j in range(4):
                        t0 = g * 4 + j
                        nc.tensor.transpose(
                            pci0[:, j * 128:(j + 1) * 128],
                            feat_g[:, j, 0:128], ident[:])
                        nc.tensor.transpose(
                            pci1[:, j * 128:(j + 1) * 128],
                            feat_g[:, j, 128:256], ident[:])
                        nc.tensor.transpose(
                            pci2[:, j * 128:(j + 1) * 128],
                            yv6n[:, j, :], ident[:])
                    ci2 = mp.tile([6, 512], F32R, name="ci2")
                    nc.vector.tensor_copy(ci2[:], pci2[:])
                    ci0 = mp.tile([128, 512], F32R, name="ci0")
                    ci1 = mp.tile([128, 512], F32R, name="ci1")
                    nc.vector.tensor_copy(ci0[:], pci0[:])
                    nc.scalar.copy(ci1[:], pci1[:])

                    ph0 = ps_mm.tile([128, 512], F32, name="ph0", tag="mm0")
                    ph1 = ps_mm.tile([128, 512], F32, name="ph1", tag="mm1")
                    chunks = [(ci0, w1[0], 128), (ci1, w1[1], 128),
                              (ci2, w1[2], 6)]
                    for ic, (ci, wt, kp) in enumerate(chunks):
                        nc.tensor.matmul(ph0[:], wt[0:kp, 0:128], ci[0:kp, :],
                                         start=(ic == 0), stop=(ic == 2))
                        nc.tensor.matmul(ph1[:], wt[0:kp, 128:256], ci[0:kp, :],
                                         start=(ic == 0), stop=(ic == 2))
                    h0 = mp.tile([128, 512], F32R, name="h0")
                    h1 = mp.tile([128, 512], F32R, name="h1")
                    nc.scalar.activation(h0[:], ph0[:], AF.Relu, bias=b1t[:, 0:1])
                    nc.scalar.activation(h1[:], ph1[:], AF.Relu, bias=b1t[:, 1:2])

                    pc0 = ps_mm.tile([128, 512], F32, name="pc0", tag="mm0")
                    pc1 = ps_mm.tile([128, 512], F32, name="pc1", tag="mm1")
                    pc2 = ps_mm.tile([3, 512], F32, name="pc2", tag="mm2")
                    for ic, (h, wt) in enumerate([(h0, w2[0]), (h1, w2[1])]):
                        nc.tensor.matmul(pc0[:], wt[:, 0:128], h[:],
                                         start=(ic == 0), stop=(ic == 1))
                        nc.tensor.matmul(pc1[:], wt[:, 128:256], h[:],
                                         start=(ic == 0), stop=(ic == 1))
                        nc.tensor.matmul(pc2[:], wt[:, 256:259], h[:],
                                         start=(ic == 0), stop=(ic == 1))
                    ct0 = mp.tile([128, 512], F32, name="ct0")
                    ct1 = mp.tile([128, 512], F32, name="ct1")
                    ct2 = mp.tile([3, 512], F32, name="ct2")
                    nc.scalar.activation(ct0[:], pc0[:], AF.Identity,
                                         bias=b2t[:, 0:1])
                    nc.scalar.activation(ct1[:], pc1[:], AF.Identity,
                                         bias=b2t[:, 1:2])
                    nc.scalar.activation(ct2[:], pc2[:], AF.Identity,
                                         bias=b2t[0:3, 2:3])

                    for j in range(4):
                        jt = g * 4 + j
                        pb = ps_bk.tile([128, 259], F32, name="pb")
                        nc.tensor.transpose(
                            pb[:, 0:128], ct0[:, j * 128:(j + 1) * 128], ident[:])
                        nc.tensor.transpose(
                            pb[:, 128:256], ct1[:, j * 128:(j + 1) * 128],
                            ident[:])
                        nc.tensor.transpose(
                            pb[:, 256:259], ct2[:, j * 128:(j + 1) * 128],
                            ident[0:3, 0:3])
                        if j % 2 == 0:
                            nc.vector.tensor_copy(cs[:, jt, :], pb[:])
                        else:
                            nc.scalar.copy(cs[:, jt, :], pb[:])

                # ---------------- stage B (whole supertile) ----------------
                wraw = cs[:, :, 0:64]
                wc = [cs[:, :, 64:128], cs[:, :, 128:192], cs[:, :, 192:256]]
                realb = cs[:, :, 256:259]
                x_v = rot_t[:, :, 0:9:3]
                y_v = rot_t[:, :, 1:9:3]

                # big scratch slots (shared tags -> bounded SBUF)
                sA = bb.tile(SH, F32, name="sA", tag="sA")
                sB = bb.tile(SH, F32, name="sB", tag="sB")
                sC = bb.tile(SH, F32, name="sC", tag="sC")
                sD = bb.tile(SH, F32, name="sD", tag="sD")
                sE = bb.tile(SH, F32, name="sE", tag="sE")
                sF = bb.tile(SH, F32, name="sF", tag="sF")
                sGa = bb.tile(SH, F32, name="sGa", tag="sGa")
                sGb = bb.tile(SH, F32, name="sGb", tag="sGb")
                RA = bb.tile([128, T, 2, K], F32, name="RA", tag="RA")
                WRA = bb.tile([128, T, 2, K], F32, name="WRA", tag="WRA")

                # small tiles
                s3a = sm.tile(S3, F32, name="s3a")
                s3b = sm.tile(S3, F32, name="s3b")
                sxx = sm.tile(S1, F32, name="sxx")
                sxy = sm.tile(S1, F32, name="sxy")
                syy = sm.tile(S1, F32, name="syy")
                nx = sm.tile(S1, F32, name="nx")
                inv_nx = sm.tile(S1, F32, name="inv_nx")
                lnx = sm.tile(S1, F32, name="lnx")
                cn = sm.tile(S1, F32, name="cn")
                s2t = sm.tile(S1, F32, name="s2t")
                xdup = sm.tile([128, T, 6], F32, name="xdup")
                ydup = sm.tile([128, T, 6], F32, name="ydup")
                cxy = sm.tile(S3, F32, name="cxy")
                v_t = sm.tile(S3, F32, name="v_t")
                xt_t = sm.tile(S3, F32, name="xt_t")
                sumw = sm.tile(S1, F32, name="sumw")
                td = sm.tile([128, T, 2], F32, name="td")
                rsum = sm.tile(S1, F32, name="rsum")
                tt = sm.tile(S1, F32, name="tt")
                dtx = sm.tile(S1, F32, name="dtx")
                args = sm.tile([128, T, 2], F32, name="args")
                u2 = sm.tile([128, T, 2], F32, name="u2")
                hpoly = sm.tile([128, T, 2], F32, name="hpoly")
                mc = sm.tile(S1, F32, name="mc")
                tsh = sm.tile(S1, F32, name="tsh")
                r_t = sm.tile(S3, F32, name="r_t")
                tx_t = sm.tile(S3, F32, name="tx_t")
                txd = sm.tile([128, T, 6], F32, name="txd")
                tz_t = sm.tile(S3, F32, name="tz_t")
                tzn = sm.tile(S1, F32, name="tzn")

                # ---- per-n prep ----
                nc.vector.tensor_mul(s3a[:], x_v, x_v)
                nc.vector.tensor_reduce(sxx[:], s3a[:], AX.X, ALU.add)
                nc.vector.tensor_mul(s3a[:], x_v, y_v)
                nc.vector.tensor_reduce(sxy[:], s3a[:], AX.X, ALU.add)
                nc.vector.tensor_mul(s3a[:], y_v, y_v)
                nc.vector.tensor_reduce(syy[:], s3a[:], AX.X, ALU.add)

                nc.scalar.activation(lnx[:], sxx[:], AF.Ln)
                nc.scalar.activation(nx[:], lnx[:], AF.Exp, scale=0.5)
                nc.scalar.activation(inv_nx[:], lnx[:], AF.Exp, scale=-0.5)

                nc.gpsimd.tensor_copy(xdup[:, :, 0:3], x_v)
                nc.gpsimd.tensor_copy(xdup[:, :, 3:6], x_v)
                nc.gpsimd.tensor_copy(ydup[:, :, 0:3], y_v)
                nc.gpsimd.tensor_copy(ydup[:, :, 3:6], y_v)
                nc.vector.tensor_mul(s3a[:], xdup[:, :, 1:4], ydup[:, :, 2:5])
                nc.vector.tensor_mul(s3b[:], xdup[:, :, 2:5], ydup[:, :, 1:4])
                nc.vector.tensor_sub(cxy[:], s3a[:], s3b[:])
                nc.vector.tensor_mul(s3a[:], cxy[:], cxy[:])
                nc.vector.tensor_reduce(cn[:], s3a[:], AX.X, ALU.add)
                nc.scalar.activation(cn[:], cn[:], AF.Ln)
                nc.scalar.activation(cn[:], cn[:], AF.Exp, scale=-0.5)
                nc.vector.tensor_mul(v_t[:], cxy[:],
                                     cn[:].unsqueeze(2)
                                     .broadcast_to(S3))

                nc.vector.tensor_mul(s3a[:], y_v,
                                     sxy[:].unsqueeze(2)
                                     .broadcast_to(S3))
                nc.vector.tensor_sub(xt_t[:], x_v, s3a[:])
                nc.vector.tensor_scalar(s2t[:], syy[:], -1.0, 2.0,
                                        ALU.mult, ALU.add)

                # ---- dots: dyw->sA, dxt->sB, dvw->sC, q->sD (sE,sF scratch)
                for eng, scr, dst, comps in (
                        (nc.vector, (sE, sF), sA,
                         [y_v[:, :, c:c + 1] for c in range(3)]),
                        (nc.vector, (sE, sF), sB,
                         [xt_t[:, :, c:c + 1] for c in range(3)]),
                        (nc.gpsimd, (sGa, sGb), sC,
                         [v_t[:, :, c:c + 1] for c in range(3)])):
                    e1, e2 = scr
                    eng.tensor_mul(e1[:], wc[0], comps[0].broadcast_to(SH))
                    eng.tensor_mul(e2[:], wc[1], comps[1].broadcast_to(SH))
                    eng.tensor_add(e1[:], e1[:], e2[:])
                    eng.tensor_mul(e2[:], wc[2], comps[2].broadcast_to(SH))
                    eng.tensor_add(dst[:], e1[:], e2[:])
                nc.scalar.activation(sE[:], wc[0], AF.Square)
                nc.scalar.activation(sF[:], wc[1], AF.Square)
                nc.vector.tensor_add(sD[:], sE[:], sF[:])
                nc.scalar.activation(sE[:], wc[2], AF.Square)
                nc.vector.tensor_add(sD[:], sD[:], sE[:])

                # ---- gamma/c/a chain ----
                nc.scalar.activation(sE[:], sA[:], AF.Square)       # dyw^2
                nc.vector.tensor_mul(sE[:], sE[:],
                                     s2t[:].unsqueeze(2)
                                     .broadcast_to(SH))
                nc.vector.tensor_sub(sE[:], sE[:], sD[:])           # -g2
                nc.scalar.activation(sA[:], sE[:], AF.Relu, scale=-1.0)  # gam
                nc.scalar.activation(sD[:], sA[:], AF.Ln)
                nc.scalar.activation(sD[:], sD[:], AF.Exp, scale=0.5)    # g
                nc.vector.tensor_scalar(sD[:], sD[:], 1.0, None, ALU.add)
                nc.vector.reciprocal_approx_fast(sE[:], sD[:])                  # rc
                nc.vector.scalar_tensor_tensor(
                    sB[:], sB[:], 0.7, sE[:], ALU.mult, ALU.mult)   # u_x
                nc.vector.scalar_tensor_tensor(
                    sC[:], sC[:], 0.7, sE[:], ALU.mult, ALU.mult)   # u_v
                nc.scalar.activation(sD[:], sE[:], AF.Square)       # rc^2
                nc.vector.scalar_tensor_tensor(
                    sD[:], sD[:], 0.49, sA[:], ALU.mult, ALU.mult)  # wn2
                nc.vector.scalar_tensor_tensor(
                    sA[:], sB[:], -2.0, sD[:], ALU.mult, ALU.add)
                nc.vector.tensor_add(sA[:], sA[:],
                                     sxx[:].unsqueeze(2)
                                     .broadcast_to(SH))             # zwn2
                nc.vector.reciprocal_approx_fast(sE[:], sA[:])                  # 1/zwn2
                nc.scalar.activation(sA[:], sD[:], AF.Identity,
                                     scale=-1.0, bias=1.0)          # 1-wn2
                nc.vector.tensor_mul(sA[:], sA[:], sE[:])           # a
                nc.vector.tensor_copy(RA[:, :, 1, :], sA[:])
                nc.vector.tensor_scalar(sD[:], sA[:], 1.0, None, ALU.add)  # 1+a

                nc.vector.tensor_mul(sE[:], sD[:],
                                     nx[:].unsqueeze(2)
                                     .broadcast_to(SH))
                nc.vector.tensor_mul(sE[:], sE[:], sC[:])           # hv2
                nc.vector.tensor_mul(sC[:], sD[:], sB[:])           # (1+a)u_x
                nc.vector.tensor_mul(sF[:], sA[:],
                                     sxx[:].unsqueeze(2)
                                     .broadcast_to(SH))
                nc.vector.tensor_sub(sF[:], sC[:], sF[:])           # hr

                # ---- atan2 half-angle; E'-based rad into RA[:, :, 0, :] ----
                nc.vector.scalar_tensor_tensor(
                    sC[:], sF[:], -1.0, sF[:], ALU.mult, ALU.max)   # |hr|
                nc.scalar.activation(sD[:], sF[:], AF.Square)
                nc.scalar.activation(sB[:], sE[:], AF.Square)
                nc.vector.tensor_add(sD[:], sD[:], sB[:])
                nc.scalar.activation(sD[:], sD[:], AF.Ln)
                nc.scalar.activation(sD[:], sD[:], AF.Exp, scale=0.5)  # norm
                nc.vector.tensor_add(sD[:], sD[:], sC[:])           # den
                nc.vector.reciprocal_approx_fast(sC[:], sD[:])
                nc.vector.tensor_mul(sD[:], sE[:], sC[:])           # qq
                nc.vector.tensor_mul(sC[:], sD[:], sD[:])           # q2
                nc.vector.tensor_scalar(
                    sB[:], sC[:], -2.0 * AT_C3, -2.0 * AT_C2, ALU.mult, ALU.add)
                nc.vector.tensor_mul(sB[:], sB[:], sC[:])
                nc.vector.scalar_tensor_tensor(
                    sB[:], sB[:], -2.0 * AT_C1, sC[:], ALU.add, ALU.mult)
                nc.vector.scalar_tensor_tensor(
                    sB[:], sB[:], -2.0 * AT_C0, sD[:], ALU.add, ALU.mult)
                # sB = E' = -2 atan(qq)

                nc.gpsimd.tensor_scalar(sC[:], sF[:], 0.0, None, ALU.is_lt)  # mx
                nc.gpsimd.tensor_scalar(sD[:], sE[:], 0.0, None, ALU.is_gt)  # my
                nc.gpsimd.tensor_mul(sF[:], sC[:], sD[:])
                nc.gpsimd.tensor_sub(sF[:], sD[:], sF[:])           # (1-mx)my
                nc.vector.tensor_mul(sE[:], sC[:], sB[:])           # mx*E'
                nc.vector.scalar_tensor_tensor(
                    sE[:], sE[:], -2.0, sB[:], ALU.mult, ALU.add)
                nc.vector.scalar_tensor_tensor(
                    sE[:], sC[:], PI, sE[:], ALU.mult, ALU.add)
                nc.vector.scalar_tensor_tensor(
                    RA[:, :, 0, :], sF[:], TWO_PI, sE[:], ALU.mult, ALU.add)

                # ---- weights + accumulations ----
                nc.scalar.activation(sA[:], wraw, AF.Exp)
                nc.scalar.activation(sA[:], sA[:], AF.Ln, bias=1.0)  # softplus
                nc.vector.tensor_reduce(sumw[:], sA[:], AX.X, ALU.add)
                nc.vector.tensor_mul(WRA[:], RA[:],
                                     sA[:].unsqueeze(2)
                                     .broadcast_to([128, T, 2, K]))
                nc.vector.tensor_reduce(td[:], WRA[:], AX.X, ALU.add)

                # ---- post ----
                nc.vector.reciprocal_approx_fast(rsum[:], sumw[:])
                nc.vector.tensor_mul(tt[:], td[:, :, 0], rsum[:])
                nc.vector.tensor_mul(dtx[:], td[:, :, 1], rsum[:])
                ldj_t = op_.tile(S1, F32, name="ldj_t")
                nc.scalar.activation(ldj_t[:], dtx[:], AF.Ln)

                nc.vector.tensor_scalar(args[:, :, 0], tt[:], -PI, None, ALU.add)
                nc.vector.tensor_scalar(mc[:], tt[:], PI / 2, None, ALU.is_ge)
                nc.vector.tensor_scalar(tsh[:], tt[:], PI / 2, None, ALU.add)
                nc.vector.scalar_tensor_tensor(
                    args[:, :, 1], mc[:], -TWO_PI, tsh[:], ALU.mult, ALU.add)

                nc.vector.tensor_mul(u2[:], args[:], args[:])
                nc.vector.tensor_scalar(
                    hpoly[:], u2[:], SIN_C[5], SIN_C[4], ALU.mult, ALU.add)
                for cf in (SIN_C[3], SIN_C[2], SIN_C[1]):
                    nc.vector.tensor_mul(hpoly[:], hpoly[:], u2[:])
                    nc.vector.tensor_scalar(hpoly[:], hpoly[:], cf, None,
                                            ALU.add)
                nc.vector.tensor_mul(hpoly[:], hpoly[:], u2[:])
                nc.vector.scalar_tensor_tensor(
                    hpoly[:], hpoly[:], SIN_C[0], args[:], ALU.add, ALU.mult)
                # hpoly[...,0] = -sin t ; hpoly[...,1] = cos t

                nc.vector.tensor_mul(r_t[:], x_v,
                                     inv_nx[:].unsqueeze(2)
                                     .broadcast_to(S3))              # x/|x|
                nc.vector.tensor_mul(tx_t[:], r_t[:],
                                     hpoly[:, :, 1:2].broadcast_to(S3))
                nc.vector.tensor_mul(s3a[:], v_t[:],
                                     hpoly[:, :, 0:1].broadcast_to(S3))
                nc.vector.tensor_add(tx_t[:], tx_t[:], s3a[:])
                nc.vector.tensor_scalar(tx_t[:], tx_t[:], -1.0, None, ALU.mult)
                # tx = -(x/|x|)cos + v sin

                nc.gpsimd.tensor_copy(txd[:, :, 0:3], tx_t[:])
                nc.gpsimd.tensor_copy(txd[:, :, 3:6], tx_t[:])
                nc.vector.tensor_mul(s3a[:], txd[:, :, 1:4], ydup[:, :, 2:5])
                nc.vector.tensor_mul(s3b[:], txd[:, :, 2:5], ydup[:, :, 1:4])
                nc.vector.tensor_sub(tz_t[:], s3a[:], s3b[:])
                nc.vector.tensor_mul(s3a[:], tz_t[:], tz_t[:])
                nc.vector.tensor_reduce(tzn[:], s3a[:], AX.X, ALU.add)
                nc.scalar.activation(tzn[:], tzn[:], AF.Ln)
                nc.scalar.activation(tzn[:], tzn[:], AF.Exp, scale=-0.5)
                nc.vector.tensor_mul(tz_t[:], tz_t[:],
                                     tzn[:].unsqueeze(2)
                                     .broadcast_to(S3))

                trot_t = op_.tile([128, T, 9], F32, name="trot_t")
                nc.vector.tensor_copy(trot_t[:, :, 0:9:3], tx_t[:])
                nc.gpsimd.tensor_copy(trot_t[:, :, 1:9:3], y_v)
                nc.scalar.copy(trot_t[:, :, 2:9:3], tz_t[:])

                tvec_t = op_.tile([128, T, 6], F32, name="tvec_t")
                nc.gpsimd.tensor_copy(tvec_t[:, :, 0:3], vec_t[:, :, 0:3])
                nc.vector.tensor_add(tvec_t[:, :, 3:6], vec_t[:, :, 3:6], realb)

                nc.sync.dma_start(
                    trot_o[base:base + ST_ROWS, :]
                    .rearrange("(p t) c -> p t c", t=T), trot_t[:])
                nc.sync.dma_start(
                    tvec_o[base:base + ST_ROWS, :]
                    .rearrange("(p t) c -> p t c", t=T), tvec_t[:])
                nc.sync.dma_start(
                    ldj_o[base:base + ST_ROWS]
                    .rearrange("(p t) -> p t", t=T), ldj_t[:])

    nc.finalize()
    return nc


def _get_nc():
    if "nc" not in _BUILD_CACHE:
        _BUILD_CACHE["nc"] = build_bass()
    return _BUILD_CACHE["nc"]


def _prep_host(W1, b1, W2, b2):
    W1p = np.concatenate([W1[3:259], W1[0:3], W1[259:262]], 0).astype(np.float32)
    idx = list(range(64))
    for c in range(3):
        for k in range(64):
            idx.append(64 + k * 3 + c)
    idx += [259, 260, 261]
    W2p = np.ascontiguousarray(W2[:, idx]).astype(np.float32)
    b2p = np.ascontiguousarray(b2[idx]).astype(np.float32)
    return W1p, np.asarray(b1, np.float32), W2p, b2p


def kernel(rotation, vector, feature, permute, W1, b1, W2, b2, trace=False):
    rotation = np.asarray(rotation)
    vector = np.asarray(vector)
    feature = np.asarray(feature)
    permute = np.asarray(permute)
    assert list(permute) == [0, 1, 2], \
        f"kernel hardcodes permute=(0,1,2), got {permute}"
    N = rotation.shape[0]
    assert N == N_TOTAL

    W1p, b1p, W2p, b2p = _prep_host(
        np.asarray(W1), np.asarray(b1), np.asarray(W2), np.asarray(b2))

    rot_flat = np.ascontiguousarray(rotation.reshape(N, 9), np.float32)
    vec_f = np.ascontiguousarray(vector, np.float32)
    feat_f = np.ascontiguousarray(feature, np.float32)

    nc = _get_nc()
    in_maps = []
    for c in range(NCORES):
        s = slice(c * R, (c + 1) * R)
        in_maps.append(dict(
            rot=rot_flat[s], vec=vec_f[s], feat=feat_f[s],
            W1p=W1p, b1d=b1p, W2p=W2p, b2d=b2p))
    res = run_bass_kernel_spmd(nc, in_maps, core_ids=list(range(NCORES)),
                               trace=trace)
    trot = np.concatenate([r["trot_o"] for r in res.results], 0).reshape(N, 3, 3)
    tvec = np.concatenate([r["tvec_o"] for r in res.results], 0)
    ldj = np.concatenate([r["ldj_o"] for r in res.results], 0)
    if trace:
        return (trot, tvec, ldj), res
    return trot, tvec, ldj


# revision 10
# speedup vs baseline: 1.3172x; 1.3172x over previous
"""MobiusFlow Trainium2 kernel: 8-core data-parallel Bass/Tile implementation.

Math (validated against the reference to fp32 precision in debug_math.py):
  x = rot[:,:,0], y = rot[:,:,1]   (NOT unit vectors)
  cond = [feature, y, vec[:, :3]] @ W1p -> relu -> @ W2p  (f32r matmuls)
  conds = [weights(64) | w_raw c-major(192) | real_b(3)]  (real_s dropped: exp*0+1)
  dyw,dxt,dvw,q = per-k dots of w_raw with y, x~=x-(x.y)y, v=norm(cross(x,y)), self
  g2 = max(q-(2-|y|^2)dyw^2, 0); rc = 1/(1+sqrt(g2)); c = 0.7*rc; wn2 = c^2 g2
  u_x = c*dxt; u_v = c*dvw; zwn2 = |x|^2 - 2u_x + wn2; a = (1-wn2)/zwn2
  hv2 = (1+a)|x| u_v  (= -hv_true*|x|);  hr = (1+a)u_x - a|x|^2  (= hr_true*|x|)
  rad = wrapped atan2(hv_true, hr) via half-angle q = hv2/(|(hv2,hr)| + |hr|):
        E' = -2*atan(q) [deg-7 poly]; rad = E' - 2*mx*E' + pi*mx + 2pi*(1-mx)*my
        with mx = [hr<0], my = [hv2>0]
  wsp = softplus = Ln(Exp(raw)+1); t = sum(wsp*rad)/sum(wsp)
  dtx = sum(wsp*a)/sum(wsp)  (Householder norm-preservation; |dz_dtheta|=1)
  tx = -(x/|x|) cos t + v sin t;  tz = norm(cross(tx, y));  ldj = Ln(dtx)
  tvec = [vec[:, :3], vec[:, 3:] + real_b]
"""
import numpy as np

import concourse.bass as bass
import concourse.bacc as bacc
import concourse.mybir as mybir
import concourse.tile as tile
from concourse import masks
from concourse.bass_utils import run_bass_kernel_spmd

F32 = mybir.dt.float32
F32R = mybir.dt.float32r
AF = mybir.ActivationFunctionType
ALU = mybir.AluOpType
AX = mybir.AxisListType

NCORES = 8
N_TOTAL = 131072
R = N_TOTAL // NCORES          # rows per core
T = 16                         # row-blocks per supertile
ST_ROWS = 128 * T              # 2048
NST = R // ST_ROWS             # 8 supertiles
NG = ST_ROWS // 512            # 4 groups of 512 rows per supertile

K = 64
PI = float(np.pi)
TWO_PI = 2.0 * PI

# atan deg-7 odd minimax on [-1,1] (fit_polys.py)
AT_C0, AT_C1, AT_C2, AT_C3 = (0.9992138034627527, -0.32117489148664036,
                              0.14626430128714862, -0.03898641853047838)
# sin deg-11 odd on [-pi, pi]
SIN_C = (0.999999603917376, -0.16666553446863705, 0.008332407595463078,
         -0.00019808740070592645, 2.6998228073890683e-06,
         -2.0366231290827938e-08)

_BUILD_CACHE = {}


def build_bass(nst=NST):
    nc = bacc.Bacc("TRN2", target_bir_lowering=False)

    rot = nc.dram_tensor("rot", [R, 9], F32, kind="ExternalInput")
    vec = nc.dram_tensor("vec", [R, 6], F32, kind="ExternalInput")
    feat = nc.dram_tensor("feat", [R, 256], F32, kind="ExternalInput")
    W1p = nc.dram_tensor("W1p", [262, 256], F32, kind="ExternalInput")
    b1d = nc.dram_tensor("b1d", [256], F32, kind="ExternalInput")
    W2p = nc.dram_tensor("W2p", [256, 259], F32, kind="ExternalInput")
    b2d = nc.dram_tensor("b2d", [259], F32, kind="ExternalInput")

    trot_o = nc.dram_tensor("trot_o", [R, 9], F32, kind="ExternalOutput")
    tvec_o = nc.dram_tensor("tvec_o", [R, 6], F32, kind="ExternalOutput")
    ldj_o = nc.dram_tensor("ldj_o", [R], F32, kind="ExternalOutput")

    SH = [128, T, K]
    S1 = [128, T]
    S3 = [128, T, 3]

    def b1c(ap_small):          # [128,T] -> [128,T,1] view
        return ap_small.unsqueeze(2)

    with tile.TileContext(nc) as tc:
        with tc.tile_pool(name="wpool", bufs=1) as wp, \
             tc.tile_pool(name="io", bufs=2) as io, \
             tc.tile_pool(name="mlp", bufs=2) as mp, \
             tc.tile_pool(name="csp", bufs=2) as csp, \
             tc.tile_pool(name="bb", bufs=2) as bb, \
             tc.tile_pool(name="sm", bufs=2) as sm, \
             tc.tile_pool(name="outp", bufs=2) as op_, \
             tc.tile_pool(name="ps_t", bufs=1, space="PSUM") as ps_t, \
             tc.tile_pool(name="ps_mm", bufs=1, space="PSUM") as ps_mm, \
             tc.tile_pool(name="ps_bk", bufs=2, space="PSUM") as ps_bk:

            # ---------------- weights / constants (once) ----------------
            ident = wp.tile([128, 128], F32)
            masks.make_identity(nc, ident[:])

            w1_f32 = [wp.tile([128, 256], F32, name="w1f_0"),
                      wp.tile([128, 256], F32, name="w1f_1"),
                      wp.tile([6, 256], F32, name="w1f_2")]
            nc.sync.dma_start(w1_f32[0][:], W1p[0:128, :])
            nc.sync.dma_start(w1_f32[1][:], W1p[128:256, :])
            nc.sync.dma_start(w1_f32[2][:], W1p[256:262, :])
            w1 = [wp.tile([128, 256], F32R, name="w1r_0"),
                  wp.tile([128, 256], F32R, name="w1r_1"),
                  wp.tile([6, 256], F32R, name="w1r_2")]
            for a, b in zip(w1_f32, w1):
                nc.vector.tensor_copy(b[:], a[:])

            w2_f32 = [wp.tile([128, 259], F32, name="w2f_0"),
                      wp.tile([128, 259], F32, name="w2f_1")]
            nc.sync.dma_start(w2_f32[0][:], W2p[0:128, :])
            nc.sync.dma_start(w2_f32[1][:], W2p[128:256, :])
            w2 = [wp.tile([128, 259], F32R, name="w2r_0"),
                  wp.tile([128, 259], F32R, name="w2r_1")]
            for a, b in zip(w2_f32, w2):
                nc.vector.tensor_copy(b[:], a[:])

            b1t = wp.tile([128, 2], F32)
            nc.sync.dma_start(b1t[:], b1d[:].rearrange("(j p) -> p j", p=128))
            b2t = wp.tile([128, 3], F32)
            nc.sync.dma_start(b2t[:, 0:2],
                              b2d[0:256].rearrange("(j p) -> p j", p=128))
            nc.sync.dma_start(b2t[0:3, 2:3], b2d[256:259].unsqueeze(1))

            for st in range(nst):
                base = st * ST_ROWS

                rot_t = io.tile([128, T, 9], F32, name="rot_t")
                nc.sync.dma_start(
                    rot_t[:], rot[base:base + ST_ROWS, :]
                    .rearrange("(p t) c -> p t c", t=T))
                vec_t = io.tile([128, T, 6], F32, name="vec_t")
                nc.sync.dma_start(
                    vec_t[:], vec[base:base + ST_ROWS, :]
                    .rearrange("(p t) c -> p t c", t=T))

                cs = csp.tile([128, T, 259], F32, name="cs")

                # ---------------- MLP per 512-row group ----------------
                for g in range(NG):
                    feat_g = io.tile([128, 4, 256], F32, name="feat_g")
                    nc.sync.dma_start(
                        feat_g[:],
                        bass.AP(tensor=feat, offset=(base + g * 4) * 256,
                                ap=[[256 * T, 128], [256, 4], [1, 256]]))
                    yv6n = mp.tile([128, 4, 6], F32, name="yv6n")
                    nc.gpsimd.tensor_copy(yv6n[:, :, 0:3],
                                          rot_t[:, g * 4:g * 4 + 4, 1:9:3])
                    nc.gpsimd.tensor_copy(yv6n[:, :, 3:6],
                                          vec_t[:, g * 4:g * 4 + 4, 0:3])

                    pci0 = ps_t.tile([128, 512], F32, name="pci0")
                    pci1 = ps_t.tile([128, 512], F32, name="pci1")
                    pci2 = ps_t.tile([6, 512], F32, name="pci2")
                    for j in range(4):
                        t0 = g * 4 + j
                        nc.tensor.transpose(
                            pci0[:, j * 128:(j + 1) * 128],
                            feat_g[:, j, 0:128], ident[:])
                        nc.tensor.transpose(
                            pci1[:, j * 128:(j + 1) * 128],
                            feat_g[:, j, 128:256], ident[:])
                        nc.tensor.transpose(
                            pci2[:, j * 128:(j + 1) * 128],
                            yv6n[:, j, :], ident[:])
                    ci2 = mp.tile([6, 512], F32R, name="ci2")
                    nc.vector.tensor_copy(ci2[:], pci2[:])
                    ci0 = mp.tile([128, 512], F32R, name="ci0")
                    ci1 = mp.tile([128, 512], F32R, name="ci1")
                    nc.vector.tensor_copy(ci0[:], pci0[:])
                    nc.scalar.copy(ci1[:], pci1[:])

                    ph0 = ps_mm.tile([128, 512], F32, name="ph0", tag="mm0")
                    ph1 = ps_mm.tile([128, 512], F32, name="ph1", tag="mm1")
                    chunks = [(ci0, w1[0], 128), (ci1, w1[1], 128),
                              (ci2, w1[2], 6)]
                    for ic, (ci, wt, kp) in enumerate(chunks):
                        nc.tensor.matmul(ph0[:], wt[0:kp, 0:128], ci[0:kp, :],
                                         start=(ic == 0), stop=(ic == 2))
                        nc.tensor.matmul(ph1[:], wt[0:kp, 128:256], ci[0:kp, :],
                                         start=(ic == 0), stop=(ic == 2))
                    h0 = mp.tile([128, 512], F32R, name="h0")
                    h1 = mp.tile([128, 512], F32R, name="h1")
                    nc.scalar.activation(h0[:], ph0[:], AF.Relu, bias=b1t[:, 0:1])
                    nc.scalar.activation(h1[:], ph1[:], AF.Relu, bias=b1t[:, 1:2])

                    pc0 = ps_mm.tile([128, 512], F32, name="pc0", tag="mm0")
                    pc1 = ps_mm.tile([128, 512], F32, name="pc1", tag="mm1")
                    pc2 = ps_mm.tile([3, 512], F32, name="pc2", tag="mm2")
                    for ic, (h, wt) in enumerate([(h0, w2[0]), (h1, w2[1])]):
                        nc.tensor.matmul(pc0[:], wt[:, 0:128], h[:],
                                         start=(ic == 0), stop=(ic == 1))
                        nc.tensor.matmul(pc1[:], wt[:, 128:256], h[:],
                                         start=(ic == 0), stop=(ic == 1))
                        nc.tensor.matmul(pc2[:], wt[:, 256:259], h[:],
                                         start=(ic == 0), stop=(ic == 1))
                    ct0 = mp.tile([128, 512], F32, name="ct0")
                    ct1 = mp.tile([128, 512], F32, name="ct1")
                    ct2 = mp.tile([3, 512], F32, name="ct2")
                    nc.scalar.activation(ct0[:], pc0[:], AF.Identity,
                                         bias=b2t[:, 0:1])
                    nc.scalar.activation(ct1[:], pc1[:], AF.Identity,
                                         bias=b2t[:, 1:2])
                    nc.scalar.activation(ct2[:], pc2[:], AF.Identity,
                                         bias=b2t[0:3, 2:3])

                    for j in range(4):
                        jt = g * 4 + j
                        pb = ps_bk.tile([128, 259], F32, name="pb")
                        nc.tensor.transpose(
                            pb[:, 0:128], ct0[:, j * 128:(j + 1) * 128], ident[:])
                        nc.tensor.transpose(
                            pb[:, 128:256], ct1[:, j * 128:(j + 1) * 128],
                            ident[:])
                        nc.tensor.transpose(
                            pb[:, 256:259], ct2[:, j * 128:(j + 1) * 128],
                            ident[0:3, 0:3])
                        if j % 2 == 0:
                            nc.vector.tensor_copy(cs[:, jt, :], pb[:])
                        else:
                            nc.scalar.copy(cs[:, jt, :], pb[:])

                # ---------------- stage B (whole supertile) ----------------
                wraw = cs[:, :, 0:64]
                wc = [cs[:, :, 64:128], cs[:, :, 128:192], cs[:, :, 192:256]]
                realb = cs[:, :, 256:259]
                x_v = rot_t[:, :, 0:9:3]
                y_v = rot_t[:, :, 1:9:3]

                # big scratch slots (shared tags -> bounded SBUF)
                sA = bb.tile(SH, F32, name="sA", tag="sA")
                sB = bb.tile(SH, F32, name="sB", tag="sB")
                sC = bb.tile(SH, F32, name="sC", tag="sC")
                sD = bb.tile(SH, F32, name="sD", tag="sD")
                sE = bb.tile(SH, F32, name="sE", tag="sE")
                sF = bb.tile(SH, F32, name="sF", tag="sF")
                RA = bb.tile([128, T, 2, K], F32, name="RA", tag="RA")
                WRA = bb.tile([128, T, 2, K], F32, name="WRA", tag="WRA")

                # small tiles
                s3a = sm.tile(S3, F32, name="s3a")
                s3b = sm.tile(S3, F32, name="s3b")
                sxx = sm.tile(S1, F32, name="sxx")
                sxy = sm.tile(S1, F32, name="sxy")
                syy = sm.tile(S1, F32, name="syy")
                nx = sm.tile(S1, F32, name="nx")
                inv_nx = sm.tile(S1, F32, name="inv_nx")
                lnx = sm.tile(S1, F32, name="lnx")
                cn = sm.tile(S1, F32, name="cn")
                s2t = sm.tile(S1, F32, name="s2t")
                xdup = sm.tile([128, T, 6], F32, name="xdup")
                ydup = sm.tile([128, T, 6], F32, name="ydup")
                cxy = sm.tile(S3, F32, name="cxy")
                v_t = sm.tile(S3, F32, name="v_t")
                xt_t = sm.tile(S3, F32, name="xt_t")
                sumw = sm.tile(S1, F32, name="sumw")
                td = sm.tile([128, T, 2], F32, name="td")
                rsum = sm.tile(S1, F32, name="rsum")
                tt = sm.tile(S1, F32, name="tt")
                dtx = sm.tile(S1, F32, name="dtx")
                args = sm.tile([128, T, 2], F32, name="args")
                u2 = sm.tile([128, T, 2], F32, name="u2")
                hpoly = sm.tile([128, T, 2], F32, name="hpoly")
                mc = sm.tile(S1, F32, name="mc")
                tsh = sm.tile(S1, F32, name="tsh")
                r_t = sm.tile(S3, F32, name="r_t")
                tx_t = sm.tile(S3, F32, name="tx_t")
                txd = sm.tile([128, T, 6], F32, name="txd")
                tz_t = sm.tile(S3, F32, name="tz_t")
                tzn = sm.tile(S1, F32, name="tzn")

                # ---- per-n prep ----
                nc.vector.tensor_mul(s3a[:], x_v, x_v)
                nc.vector.tensor_reduce(sxx[:], s3a[:], AX.X, ALU.add)
                nc.vector.tensor_mul(s3a[:], x_v, y_v)
                nc.vector.tensor_reduce(sxy[:], s3a[:], AX.X, ALU.add)
                nc.vector.tensor_mul(s3a[:], y_v, y_v)
                nc.vector.tensor_reduce(syy[:], s3a[:], AX.X, ALU.add)

                nc.scalar.activation(lnx[:], sxx[:], AF.Ln)
                nc.scalar.activation(nx[:], lnx[:], AF.Exp, scale=0.5)
                nc.scalar.activation(inv_nx[:], lnx[:], AF.Exp, scale=-0.5)

                nc.gpsimd.tensor_copy(xdup[:, :, 0:3], x_v)
                nc.gpsimd.tensor_copy(xdup[:, :, 3:6], x_v)
                nc.gpsimd.tensor_copy(ydup[:, :, 0:3], y_v)
                nc.gpsimd.tensor_copy(ydup[:, :, 3:6], y_v)
                nc.vector.tensor_mul(s3a[:], xdup[:, :, 1:4], ydup[:, :, 2:5])
                nc.vector.tensor_mul(s3b[:], xdup[:, :, 2:5], ydup[:, :, 1:4])
                nc.vector.tensor_sub(cxy[:], s3a[:], s3b[:])
                nc.vector.tensor_mul(s3a[:], cxy[:], cxy[:])
                nc.vector.tensor_reduce(cn[:], s3a[:], AX.X, ALU.add)
                nc.scalar.activation(cn[:], cn[:], AF.Ln)
                nc.scalar.activation(cn[:], cn[:], AF.Exp, scale=-0.5)
                nc.vector.tensor_mul(v_t[:], cxy[:],
                                     cn[:].unsqueeze(2)
                                     .broadcast_to(S3))

                nc.vector.tensor_mul(s3a[:], y_v,
                                     sxy[:].unsqueeze(2)
                                     .broadcast_to(S3))
                nc.vector.tensor_sub(xt_t[:], x_v, s3a[:])
                nc.vector.tensor_scalar(s2t[:], syy[:], -1.0, 2.0,
                                        ALU.mult, ALU.add)

                # ---- dots: dyw->sA, dxt->sB, dvw->sC, q->sD (sE,sF scratch)
                for dst, comps in ((sA, [y_v[:, :, c:c + 1] for c in range(3)]),
                                   (sB, [xt_t[:, :, c:c + 1] for c in range(3)]),
                                   (sC, [v_t[:, :, c:c + 1] for c in range(3)])):
                    nc.vector.tensor_mul(sE[:], wc[0], comps[0].broadcast_to(SH))
                    nc.vector.tensor_mul(sF[:], wc[1], comps[1].broadcast_to(SH))
                    nc.vector.tensor_add(sE[:], sE[:], sF[:])
                    nc.vector.tensor_mul(sF[:], wc[2], comps[2].broadcast_to(SH))
                    nc.vector.tensor_add(dst[:], sE[:], sF[:])
                nc.scalar.activation(sE[:], wc[0], AF.Square)
                nc.scalar.activation(sF[:], wc[1], AF.Square)
                nc.vector.tensor_add(sD[:], sE[:], sF[:])
                nc.scalar.activation(sE[:], wc[2], AF.Square)
                nc.vector.tensor_add(sD[:], sD[:], sE[:])

                # ---- gamma/c/a chain ----
                nc.scalar.activation(sE[:], sA[:], AF.Square)       # dyw^2
                nc.vector.tensor_mul(sE[:], sE[:],
                                     s2t[:].unsqueeze(2)
                                     .broadcast_to(SH))
                nc.vector.tensor_sub(sE[:], sE[:], sD[:])           # -g2
                nc.scalar.activation(sA[:], sE[:], AF.Relu, scale=-1.0)  # gam
                nc.scalar.activation(sD[:], sA[:], AF.Ln)
                nc.scalar.activation(sD[:], sD[:], AF.Exp, scale=0.5)    # g
                nc.vector.tensor_scalar(sD[:], sD[:], 1.0, None, ALU.add)
                nc.vector.reciprocal_approx_fast(sE[:], sD[:])                  # rc
                nc.vector.scalar_tensor_tensor(
                    sB[:], sB[:], 0.7, sE[:], ALU.mult, ALU.mult)   # u_x
                nc.vector.scalar_tensor_tensor(
                    sC[:], sC[:], 0.7, sE[:], ALU.mult, ALU.mult)   # u_v
                nc.scalar.activation(sD[:], sE[:], AF.Square)       # rc^2
                nc.vector.scalar_tensor_tensor(
                    sD[:], sD[:], 0.49, sA[:], ALU.mult, ALU.mult)  # wn2
                nc.vector.scalar_tensor_tensor(
                    sA[:], sB[:], -2.0, sD[:], ALU.mult, ALU.add)
                nc.vector.tensor_add(sA[:], sA[:],
                                     sxx[:].unsqueeze(2)
                                     .broadcast_to(SH))             # zwn2
                nc.vector.reciprocal_approx_fast(sE[:], sA[:])                  # 1/zwn2
                nc.scalar.activation(sA[:], sD[:], AF.Identity,
                                     scale=-1.0, bias=1.0)          # 1-wn2
                nc.vector.tensor_mul(sA[:], sA[:], sE[:])           # a
                nc.vector.tensor_copy(RA[:, :, 1, :], sA[:])
                nc.vector.tensor_scalar(sD[:], sA[:], 1.0, None, ALU.add)  # 1+a

                nc.vector.tensor_mul(sE[:], sD[:],
                                     nx[:].unsqueeze(2)
                                     .broadcast_to(SH))
                nc.vector.tensor_mul(sE[:], sE[:], sC[:])           # hv2
                nc.vector.tensor_mul(sC[:], sD[:], sB[:])           # (1+a)u_x
                nc.vector.tensor_mul(sF[:], sA[:],
                                     sxx[:].unsqueeze(2)
                                     .broadcast_to(SH))
                nc.vector.tensor_sub(sF[:], sC[:], sF[:])           # hr

                # ---- atan2 half-angle; E'-based rad into RA[:, :, 0, :] ----
                nc.vector.scalar_tensor_tensor(
                    sC[:], sF[:], -1.0, sF[:], ALU.mult, ALU.max)   # |hr|
                nc.scalar.activation(sD[:], sF[:], AF.Square)
                nc.scalar.activation(sB[:], sE[:], AF.Square)
                nc.vector.tensor_add(sD[:], sD[:], sB[:])
                nc.scalar.activation(sD[:], sD[:], AF.Ln)
                nc.scalar.activation(sD[:], sD[:], AF.Exp, scale=0.5)  # norm
                nc.vector.tensor_add(sD[:], sD[:], sC[:])           # den
                nc.vector.reciprocal_approx_fast(sC[:], sD[:])
                nc.vector.tensor_mul(sD[:], sE[:], sC[:])           # qq
                nc.vector.tensor_mul(sC[:], sD[:], sD[:])           # q2
                nc.vector.tensor_scalar(
                    sB[:], sC[:], -2.0 * AT_C3, -2.0 * AT_C2, ALU.mult, ALU.add)
                nc.vector.tensor_mul(sB[:], sB[:], sC[:])
                nc.vector.scalar_tensor_tensor(
                    sB[:], sB[:], -2.0 * AT_C1, sC[:], ALU.add, ALU.mult)
                nc.vector.scalar_tensor_tensor(
                    sB[:], sB[:], -2.0 * AT_C0, sD[:], ALU.add, ALU.mult)
                # sB = E' = -2 atan(qq)

                nc.vector.tensor_scalar(sC[:], sF[:], 0.0, None, ALU.is_lt)  # mx
                nc.vector.tensor_scalar(sD[:], sE[:], 0.0, None, ALU.is_gt)  # my
                nc.vector.tensor_mul(sF[:], sC[:], sD[:])
                nc.vector.tensor_sub(sF[:], sD[:], sF[:])           # (1-mx)my
                nc.vector.tensor_mul(sE[:], sC[:], sB[:])           # mx*E'
                nc.vector.scalar_tensor_tensor(
                    sE[:], sE[:], -2.0, sB[:], ALU.mult, ALU.add)
                nc.vector.scalar_tensor_tensor(
                    sE[:], sC[:], PI, sE[:], ALU.mult, ALU.add)
                nc.vector.scalar_tensor_tensor(
                    RA[:, :, 0, :], sF[:], TWO_PI, sE[:], ALU.mult, ALU.add)

                # ---- weights + accumulations ----
                nc.scalar.activation(sA[:], wraw, AF.Exp)
                nc.scalar.activation(sA[:], sA[:], AF.Ln, bias=1.0)  # softplus
                nc.vector.tensor_reduce(sumw[:], sA[:], AX.X, ALU.add)
                nc.vector.tensor_mul(WRA[:], RA[:],
                                     sA[:].unsqueeze(2)
                                     .broadcast_to([128, T, 2, K]))
                nc.vector.tensor_reduce(td[:], WRA[:], AX.X, ALU.add)

                # ---- post ----
                nc.vector.reciprocal_approx_fast(rsum[:], sumw[:])
                nc.vector.tensor_mul(tt[:], td[:, :, 0], rsum[:])
                nc.vector.tensor_mul(dtx[:], td[:, :, 1], rsum[:])
                ldj_t = op_.tile(S1, F32, name="ldj_t")
                nc.scalar.activation(ldj_t[:], dtx[:], AF.Ln)

                nc.vector.tensor_scalar(args[:, :, 0], tt[:], -PI, None, ALU.add)
                nc.vector.tensor_scalar(mc[:], tt[:], PI / 2, None, ALU.is_ge)
                nc.vector.tensor_scalar(tsh[:], tt[:], PI / 2, None, ALU.add)
                nc.vector.scalar_tensor_tensor(
                    args[:, :, 1], mc[:], -TWO_PI, tsh[:], ALU.mult, ALU.add)

                nc.vector.tensor_mul(u2[:], args[:], args[:])
                nc.vector.tensor_scalar(
                    hpoly[:], u2[:], SIN_C[5], SIN_C[4], ALU.mult, ALU.add)
                for cf in (SIN_C[3], SIN_C[2], SIN_C[1]):
                    nc.vector.tensor_mul(hpoly[:], hpoly[:], u2[:])
                    nc.vector.tensor_scalar(hpoly[:], hpoly[:], cf, None,
                                            ALU.add)
                nc.vector.tensor_mul(hpoly[:], hpoly[:], u2[:])
                nc.vector.scalar_tensor_tensor(
                    hpoly[:], hpoly[:], SIN_C[0], args[:], ALU.add, ALU.mult)
                # hpoly[...,0] = -sin t ; hpoly[...,1] = cos t

                nc.vector.tensor_mul(r_t[:], x_v,
                                     inv_nx[:].unsqueeze(2)
                                     .broadcast_to(S3))              # x/|x|
                nc.vector.tensor_mul(tx_t[:], r_t[:],
                                     hpoly[:, :, 1:2].broadcast_to(S3))
                nc.vector.tensor_mul(s3a[:], v_t[:],
                                     hpoly[:, :, 0:1].broadcast_to(S3))
                nc.vector.tensor_add(tx_t[:], tx_t[:], s3a[:])
                nc.vector.tensor_scalar(tx_t[:], tx_t[:], -1.0, None, ALU.mult)
                # tx = -(x/|x|)cos + v sin

                nc.gpsimd.tensor_copy(txd[:, :, 0:3], tx_t[:])
                nc.gpsimd.tensor_copy(txd[:, :, 3:6], tx_t[:])
                nc.vector.tensor_mul(s3a[:], txd[:, :, 1:4], ydup[:, :, 2:5])
                nc.vector.tensor_mul(s3b[:], txd[:, :, 2:5], ydup[:, :, 1:4])
                nc.vector.tensor_sub(tz_t[:], s3a[:], s3b[:])
                nc.vector.tensor_mul(s3a[:], tz_t[:], tz_t[:])
                nc.vector.tensor_reduce(tzn[:], s3a[:], AX.X, ALU.add)
                nc.scalar.activation(tzn[:], tzn[:], AF.Ln)
                nc.scalar.activation(tzn[:], tzn[:], AF.Exp, scale=-0.5)
                nc.vector.tensor_mul(tz_t[:], tz_t[:],
                                     tzn[:].unsqueeze(2)
                                     .broadcast_to(S3))

                trot_t = op_.tile([128, T, 9], F32, name="trot_t")
                nc.vector.tensor_copy(trot_t[:, :, 0:9:3], tx_t[:])
                nc.gpsimd.tensor_copy(trot_t[:, :, 1:9:3], y_v)
                nc.scalar.copy(trot_t[:, :, 2:9:3], tz_t[:])

                tvec_t = op_.tile([128, T, 6], F32, name="tvec_t")
                nc.gpsimd.tensor_copy(tvec_t[:, :, 0:3], vec_t[:, :, 0:3])
                nc.vector.tensor_add(tvec_t[:, :, 3:6], vec_t[:, :, 3:6], realb)

                nc.sync.dma_start(
                    trot_o[base:base + ST_ROWS, :]
                    .rearrange("(p t) c -> p t c", t=T), trot_t[:])
                nc.sync.dma_start(
                    tvec_o[base:base + ST_ROWS, :]
                    .rearrange("(p t) c -> p t c", t=T), tvec_t[:])
                nc.sync.dma_start(
                    ldj_o[base:base + ST_ROWS]
                    .rearrange("(p t) -> p t", t=T), ldj_t[:])

    nc.finalize()
    return nc


def _get_nc():
    if "nc" not in _BUILD_CACHE:
        _BUILD_CACHE["nc"] = build_bass()
    return _BUILD_CACHE["nc"]


def _prep_host(W1, b1, W2, b2):
    W1p = np.concatenate([W1[3:259], W1[0:3], W1[259:262]], 0).astype(np.float32)
    idx = list(range(64))
    for c in range(3):
        for k in range(64):
            idx.append(64 + k * 3 + c)
    idx += [259, 260, 261]
    W2p = np.ascontiguousarray(W2[:, idx]).astype(np.float32)
    b2p = np.ascontiguousarray(b2[idx]).astype(np.float32)
    return W1p, np.asarray(b1, np.float32), W2p, b2p


def kernel(rotation, vector, feature, permute, W1, b1, W2, b2, trace=False):
    rotation = np.asarray(rotation)
    vector = np.asarray(vector)
    feature = np.asarray(feature)
    permute = np.asarray(permute)
    assert list(permute) == [0, 1, 2], \
        f"kernel hardcodes permute=(0,1,2), got {permute}"
    N = rotation.shape[0]
    assert N == N_TOTAL

    W1p, b1p, W2p, b2p = _prep_host(
        np.asarray(W1), np.asarray(b1), np.asarray(W2), np.asarray(b2))

    rot_flat = np.ascontiguousarray(rotation.reshape(N, 9), np.float32)
    vec_f = np.ascontiguousarray(vector, np.float32)
    feat_f = np.ascontiguousarray(feature, np.float32)

    nc = _get_nc()
    in_maps = []
    for c in range(NCORES):
        s = slice(c * R, (c + 1) * R)
        in_maps.append(dict(
            rot=rot_flat[s], vec=vec_f[s], feat=feat_f[s],
            W1p=W1p, b1d=b1p, W2p=W2p, b2d=b2p))
    res = run_bass_kernel_spmd(nc, in_maps, core_ids=list(range(NCORES)),
                               trace=trace)
    trot = np.concatenate([r["trot_o"] for r in res.results], 0).reshape(N, 3, 3)
    tvec = np.concatenate([r["tvec_o"] for r in res.results], 0)
    ldj = np.concatenate([r["ldj_o"] for r in res.results], 0)
    if trace:
        return (trot, tvec, ldj), res
    return trot, tvec, ldj


# revision 12
# speedup vs baseline: 1.3649x; 1.0362x over previous
"""MobiusFlow Trainium2 kernel: 8-core data-parallel Bass/Tile implementation.

Math (validated against the reference to fp32 precision in debug_math.py):
  x = rot[:,:,0], y = rot[:,:,1]   (NOT unit vectors)
  cond = [feature, y, vec[:, :3]] @ W1p -> relu -> @ W2p  (f32r matmuls)
  conds = [weights(64) | w_raw c-major(192) | real_b(3)]  (real_s dropped: exp*0+1)
  dyw,dxt,dvw,q = per-k dots of w_raw with y, x~=x-(x.y)y, v=norm(cross(x,y)), self
  g2 = max(q-(2-|y|^2)dyw^2, 0); rc = 1/(1+sqrt(g2)); c = 0.7*rc; wn2 = c^2 g2
  u_x = c*dxt; u_v = c*dvw; zwn2 = |x|^2 - 2u_x + wn2; a = (1-wn2)/zwn2
  hv2 = (1+a)|x| u_v  (= -hv_true*|x|);  hr = (1+a)u_x - a|x|^2  (= hr_true*|x|)
  rad = wrapped atan2(hv_true, hr) via half-angle q = hv2/(|(hv2,hr)| + |hr|):
        E' = -2*atan(q) [deg-7 poly]; rad = E' - 2*mx*E' + pi*mx + 2pi*(1-mx)*my
        with mx = [hr<0], my = [hv2>0]
  wsp = softplus = Ln(Exp(raw)+1); t = sum(wsp*rad)/sum(wsp)
  dtx = sum(wsp*a)/sum(wsp)  (Householder norm-preservation; |dz_dtheta|=1)
  tx = -(x/|x|) cos t + v sin t;  tz = norm(cross(tx, y));  ldj = Ln(dtx)
  tvec = [vec[:, :3], vec[:, 3:] + real_b]
"""
import numpy as np

import concourse.bass as bass
import concourse.bacc as bacc
import concourse.mybir as mybir
import concourse.tile as tile
from concourse import masks
from concourse.bass_utils import run_bass_kernel_spmd

F32 = mybir.dt.float32
F32R = mybir.dt.float32r
AF = mybir.ActivationFunctionType
ALU = mybir.AluOpType
AX = mybir.AxisListType

NCORES = 8
N_TOTAL = 131072
R = N_TOTAL // NCORES          # rows per core
T = 16                         # row-blocks per supertile
ST_ROWS = 128 * T              # 2048
NST = R // ST_ROWS             # 8 supertiles
NG = ST_ROWS // 512            # 4 groups of 512 rows per supertile

K = 64
PI = float(np.pi)
TWO_PI = 2.0 * PI

# atan deg-7 odd minimax on [-1,1] (fit_polys.py)
AT_C0, AT_C1, AT_C2, AT_C3 = (0.9992138034627527, -0.32117489148664036,
                              0.14626430128714862, -0.03898641853047838)
# sin deg-11 odd on [-pi, pi]
SIN_C = (0.999999603917376, -0.16666553446863705, 0.008332407595463078,
         -0.00019808740070592645, 2.6998228073890683e-06,
         -2.0366231290827938e-08)

_BUILD_CACHE = {}


def build_bass(nst=NST):
    nc = bacc.Bacc("TRN2", target_bir_lowering=False)

    rot = nc.dram_tensor("rot", [R, 9], F32, kind="ExternalInput")
    vec = nc.dram_tensor("vec", [R, 6], F32, kind="ExternalInput")
    feat = nc.dram_tensor("feat", [R, 256], F32, kind="ExternalInput")
    W1p = nc.dram_tensor("W1p", [262, 256], F32, kind="ExternalInput")
    b1d = nc.dram_tensor("b1d", [256], F32, kind="ExternalInput")
    W2p = nc.dram_tensor("W2p", [256, 259], F32, kind="ExternalInput")
    b2d = nc.dram_tensor("b2d", [259], F32, kind="ExternalInput")

    trot_o = nc.dram_tensor("trot_o", [R, 9], F32, kind="ExternalOutput")
    tvec_o = nc.dram_tensor("tvec_o", [R, 6], F32, kind="ExternalOutput")
    ldj_o = nc.dram_tensor("ldj_o", [R], F32, kind="ExternalOutput")

    SH = [128, T, K]
    S1 = [128, T]
    S3 = [128, T, 3]

    def b1c(ap_small):          # [128,T] -> [128,T,1] view
        return ap_small.unsqueeze(2)

    with tile.TileContext(nc) as tc:
        with tc.tile_pool(name="wpool", bufs=1) as wp, \
             tc.tile_pool(name="io", bufs=2) as io, \
             tc.tile_pool(name="mlp", bufs=2) as mp, \
             tc.tile_pool(name="csp", bufs=2) as csp, \
             tc.tile_pool(name="bb", bufs=2) as bb, \
             tc.tile_pool(name="sm", bufs=2) as sm, \
             tc.tile_pool(name="outp", bufs=2) as op_, \
             tc.tile_pool(name="ps_t", bufs=1, space="PSUM") as ps_t, \
             tc.tile_pool(name="ps_mm", bufs=1, space="PSUM") as ps_mm, \
             tc.tile_pool(name="ps_bk", bufs=2, space="PSUM") as ps_bk:

            # ---------------- weights / constants (once) ----------------
            ident = wp.tile([128, 128], F32)
            masks.make_identity(nc, ident[:])

            w1_f32 = [wp.tile([128, 256], F32, name="w1f_0"),
                      wp.tile([128, 256], F32, name="w1f_1"),
                      wp.tile([6, 256], F32, name="w1f_2")]
            nc.sync.dma_start(w1_f32[0][:], W1p[0:128, :])
            nc.sync.dma_start(w1_f32[1][:], W1p[128:256, :])
            nc.sync.dma_start(w1_f32[2][:], W1p[256:262, :])
            w1 = [wp.tile([128, 256], F32R, name="w1r_0"),
                  wp.tile([128, 256], F32R, name="w1r_1"),
                  wp.tile([6, 256], F32R, name="w1r_2")]
            for a, b in zip(w1_f32, w1):
                nc.vector.tensor_copy(b[:], a[:])

            w2_f32 = [wp.tile([128, 259], F32, name="w2f_0"),
                      wp.tile([128, 259], F32, name="w2f_1")]
            nc.sync.dma_start(w2_f32[0][:], W2p[0:128, :])
            nc.sync.dma_start(w2_f32[1][:], W2p[128:256, :])
            w2 = [wp.tile([128, 259], F32R, name="w2r_0"),
                  wp.tile([128, 259], F32R, name="w2r_1")]
            for a, b in zip(w2_f32, w2):
                nc.vector.tensor_copy(b[:], a[:])

            b1t = wp.tile([128, 2], F32)
            nc.sync.dma_start(b1t[:], b1d[:].rearrange("(j p) -> p j", p=128))
            b2t = wp.tile([128, 3], F32)
            nc.sync.dma_start(b2t[:, 0:2],
                              b2d[0:256].rearrange("(j p) -> p j", p=128))
            nc.sync.dma_start(b2t[0:3, 2:3], b2d[256:259].unsqueeze(1))

            for st in range(nst):
                base = st * ST_ROWS

                rot_t = io.tile([128, T, 9], F32, name="rot_t")
                nc.sync.dma_start(
                    rot_t[:], rot[base:base + ST_ROWS, :]
                    .rearrange("(p t) c -> p t c", t=T))
                vec_t = io.tile([128, T, 6], F32, name="vec_t")
                nc.sync.dma_start(
                    vec_t[:], vec[base:base + ST_ROWS, :]
                    .rearrange("(p t) c -> p t c", t=T))

                cs = csp.tile([128, T, 259], F32, name="cs")

                # ---------------- MLP per 512-row group ----------------
                for g in range(NG):
                    feat_g = io.tile([128, 4, 256], F32, name="feat_g")
                    nc.sync.dma_start(
                        feat_g[:],
                        bass.AP(tensor=feat, offset=(base + g * 4) * 256,
                                ap=[[256 * T, 128], [256, 4], [1, 256]]))
                    yv6n = mp.tile([128, 4, 6], F32, name="yv6n")
                    nc.gpsimd.tensor_copy(yv6n[:, :, 0:3],
                                          rot_t[:, g * 4:g * 4 + 4, 1:9:3])
                    nc.gpsimd.tensor_copy(yv6n[:, :, 3:6],
                                          vec_t[:, g * 4:g * 4 + 4, 0:3])

                    pci0 = ps_t.tile([128, 512], F32, name="pci0")
                    pci1 = ps_t.tile([128, 512], F32, name="pci1")
                    pci2 = ps_t.tile([6, 512], F32, name="pci2")
                    for j in range(4):
                        t0 = g * 4 + j
                        nc.tensor.transpose(
                            pci0[:, j * 128:(j + 1) * 128],
                            feat_g[:, j, 0:128], ident[:])
                        nc.tensor.transpose(
                            pci1[:, j * 128:(j + 1) * 128],
                            feat_g[:, j, 128:256], ident[:])
                        nc.tensor.transpose(
                            pci2[:, j * 128:(j + 1) * 128],
                            yv6n[:, j, :], ident[:])
                    ci2 = mp.tile([6, 512], F32R, name="ci2")
                    nc.scalar.copy(ci2[:], pci2[:])
                    ci0 = mp.tile([128, 512], F32R, name="ci0")
                    ci1 = mp.tile([128, 512], F32R, name="ci1")
                    nc.vector.tensor_copy(ci0[:], pci0[:])
                    nc.scalar.copy(ci1[:], pci1[:])

                    ph0 = ps_mm.tile([128, 512], F32, name="ph0", tag="mm0")
                    ph1 = ps_mm.tile([128, 512], F32, name="ph1", tag="mm1")
                    chunks = [(ci0, w1[0], 128), (ci1, w1[1], 128),
                              (ci2, w1[2], 6)]
                    for ic, (ci, wt, kp) in enumerate(chunks):
                        nc.tensor.matmul(ph0[:], wt[0:kp, 0:128], ci[0:kp, :],
                                         start=(ic == 0), stop=(ic == 2))
                        nc.tensor.matmul(ph1[:], wt[0:kp, 128:256], ci[0:kp, :],
                                         start=(ic == 0), stop=(ic == 2))
                    h0 = mp.tile([128, 512], F32R, name="h0")
                    h1 = mp.tile([128, 512], F32R, name="h1")
                    nc.scalar.activation(h0[:], ph0[:], AF.Relu, bias=b1t[:, 0:1])
                    nc.scalar.activation(h1[:], ph1[:], AF.Relu, bias=b1t[:, 1:2])

                    pc0 = ps_mm.tile([128, 512], F32, name="pc0", tag="mm0")
                    pc1 = ps_mm.tile([128, 512], F32, name="pc1", tag="mm1")
                    pc2 = ps_mm.tile([3, 512], F32, name="pc2", tag="mm2")
                    for ic, (h, wt) in enumerate([(h0, w2[0]), (h1, w2[1])]):
                        nc.tensor.matmul(pc0[:], wt[:, 0:128], h[:],
                                         start=(ic == 0), stop=(ic == 1))
                        nc.tensor.matmul(pc1[:], wt[:, 128:256], h[:],
                                         start=(ic == 0), stop=(ic == 1))
                        nc.tensor.matmul(pc2[:], wt[:, 256:259], h[:],
                                         start=(ic == 0), stop=(ic == 1))
                    ct0 = mp.tile([128, 512], F32, name="ct0")
                    ct1 = mp.tile([128, 512], F32, name="ct1")
                    ct2 = mp.tile([3, 512], F32, name="ct2")
                    nc.scalar.activation(ct0[:], pc0[:], AF.Identity,
                                         bias=b2t[:, 0:1])
                    nc.scalar.activation(ct1[:], pc1[:], AF.Identity,
                                         bias=b2t[:, 1:2])
                    nc.scalar.activation(ct2[:], pc2[:], AF.Identity,
                                         bias=b2t[0:3, 2:3])

                    for j in range(4):
                        jt = g * 4 + j
                        pb = ps_bk.tile([128, 259], F32, name="pb")
                        nc.tensor.transpose(
                            pb[:, 0:128], ct0[:, j * 128:(j + 1) * 128], ident[:])
                        nc.tensor.transpose(
                            pb[:, 128:256], ct1[:, j * 128:(j + 1) * 128],
                            ident[:])
                        nc.tensor.transpose(
                            pb[:, 256:259], ct2[:, j * 128:(j + 1) * 128],
                            ident[0:3, 0:3])
                        if j % 2 == 0:
                            nc.vector.tensor_copy(cs[:, jt, :], pb[:])
                        else:
                            nc.scalar.copy(cs[:, jt, :], pb[:])

                # ---------------- stage B (whole supertile) ----------------
                wraw = cs[:, :, 0:64]
                wc = [cs[:, :, 64:128], cs[:, :, 128:192], cs[:, :, 192:256]]
                realb = cs[:, :, 256:259]
                x_v = rot_t[:, :, 0:9:3]
                y_v = rot_t[:, :, 1:9:3]

                # big scratch slots (shared tags -> bounded SBUF)
                sA = bb.tile(SH, F32, name="sA", tag="sA")
                sB = bb.tile(SH, F32, name="sB", tag="sB")
                sC = bb.tile(SH, F32, name="sC", tag="sC")
                sD = bb.tile(SH, F32, name="sD", tag="sD")
                sE = bb.tile(SH, F32, name="sE", tag="sE")
                sF = bb.tile(SH, F32, name="sF", tag="sF")
                mx8 = bb.tile(SH, mybir.dt.uint8, name="mx8", tag="mx8")
                my8 = bb.tile(SH, mybir.dt.uint8, name="my8", tag="my8")
                RA = bb.tile([128, T, 2, K], F32, name="RA", tag="RA")
                WRA = bb.tile([128, T, 2, K], F32, name="WRA", tag="WRA")

                # small tiles
                s3a = sm.tile(S3, F32, name="s3a")
                s3b = sm.tile(S3, F32, name="s3b")
                sxx = sm.tile(S1, F32, name="sxx")
                sxy = sm.tile(S1, F32, name="sxy")
                syy = sm.tile(S1, F32, name="syy")
                nx = sm.tile(S1, F32, name="nx")
                inv_nx = sm.tile(S1, F32, name="inv_nx")
                lnx = sm.tile(S1, F32, name="lnx")
                cn = sm.tile(S1, F32, name="cn")
                s2t = sm.tile(S1, F32, name="s2t")
                xdup = sm.tile([128, T, 6], F32, name="xdup")
                ydup = sm.tile([128, T, 6], F32, name="ydup")
                cxy = sm.tile(S3, F32, name="cxy")
                v_t = sm.tile(S3, F32, name="v_t")
                xt_t = sm.tile(S3, F32, name="xt_t")
                sumw = sm.tile(S1, F32, name="sumw")
                td = sm.tile([128, T, 2], F32, name="td")
                rsum = sm.tile(S1, F32, name="rsum")
                tt = sm.tile(S1, F32, name="tt")
                dtx = sm.tile(S1, F32, name="dtx")
                args = sm.tile([128, T, 2], F32, name="args")
                u2 = sm.tile([128, T, 2], F32, name="u2")
                hpoly = sm.tile([128, T, 2], F32, name="hpoly")
                mc = sm.tile(S1, F32, name="mc")
                tsh = sm.tile(S1, F32, name="tsh")
                r_t = sm.tile(S3, F32, name="r_t")
                tx_t = sm.tile(S3, F32, name="tx_t")
                txd = sm.tile([128, T, 6], F32, name="txd")
                tz_t = sm.tile(S3, F32, name="tz_t")
                tzn = sm.tile(S1, F32, name="tzn")

                # ---- per-n prep ----
                nc.vector.tensor_mul(s3a[:], x_v, x_v)
                nc.vector.tensor_reduce(sxx[:], s3a[:], AX.X, ALU.add)
                nc.vector.tensor_mul(s3a[:], x_v, y_v)
                nc.vector.tensor_reduce(sxy[:], s3a[:], AX.X, ALU.add)
                nc.vector.tensor_mul(s3a[:], y_v, y_v)
                nc.vector.tensor_reduce(syy[:], s3a[:], AX.X, ALU.add)

                nc.scalar.activation(lnx[:], sxx[:], AF.Ln)
                nc.scalar.activation(nx[:], lnx[:], AF.Exp, scale=0.5)
                nc.scalar.activation(inv_nx[:], lnx[:], AF.Exp, scale=-0.5)

                nc.gpsimd.tensor_copy(xdup[:, :, 0:3], x_v)
                nc.gpsimd.tensor_copy(xdup[:, :, 3:6], x_v)
                nc.gpsimd.tensor_copy(ydup[:, :, 0:3], y_v)
                nc.gpsimd.tensor_copy(ydup[:, :, 3:6], y_v)
                nc.vector.tensor_mul(s3a[:], xdup[:, :, 1:4], ydup[:, :, 2:5])
                nc.vector.tensor_mul(s3b[:], xdup[:, :, 2:5], ydup[:, :, 1:4])
                nc.vector.tensor_sub(cxy[:], s3a[:], s3b[:])
                nc.vector.tensor_mul(s3a[:], cxy[:], cxy[:])
                nc.vector.tensor_reduce(cn[:], s3a[:], AX.X, ALU.add)
                nc.scalar.activation(cn[:], cn[:], AF.Ln)
                nc.scalar.activation(cn[:], cn[:], AF.Exp, scale=-0.5)
                nc.vector.tensor_mul(v_t[:], cxy[:],
                                     cn[:].unsqueeze(2)
                                     .broadcast_to(S3))

                nc.vector.tensor_mul(s3a[:], y_v,
                                     sxy[:].unsqueeze(2)
                                     .broadcast_to(S3))
                nc.vector.tensor_sub(xt_t[:], x_v, s3a[:])
                nc.vector.tensor_scalar(s2t[:], syy[:], -1.0, 2.0,
                                        ALU.mult, ALU.add)

                # ---- dots: dyw->sA, dxt->sB, dvw->sC, q->sD (sE,sF scratch)
                for dst, comps in ((sA, [y_v[:, :, c:c + 1] for c in range(3)]),
                                   (sB, [xt_t[:, :, c:c + 1] for c in range(3)]),
                                   (sC, [v_t[:, :, c:c + 1] for c in range(3)])):
                    nc.vector.tensor_mul(sE[:], wc[0], comps[0].broadcast_to(SH))
                    nc.vector.tensor_mul(sF[:], wc[1], comps[1].broadcast_to(SH))
                    nc.vector.tensor_add(sE[:], sE[:], sF[:])
                    nc.vector.tensor_mul(sF[:], wc[2], comps[2].broadcast_to(SH))
                    nc.vector.tensor_add(dst[:], sE[:], sF[:])
                nc.scalar.activation(sE[:], wc[0], AF.Square)
                nc.scalar.activation(sF[:], wc[1], AF.Square)
                nc.vector.tensor_add(sD[:], sE[:], sF[:])
                nc.scalar.activation(sE[:], wc[2], AF.Square)
                nc.vector.tensor_add(sD[:], sD[:], sE[:])

                # ---- gamma/c/a chain ----
                nc.scalar.activation(sE[:], sA[:], AF.Square)       # dyw^2
                nc.vector.tensor_mul(sE[:], sE[:],
                                     s2t[:].unsqueeze(2)
                                     .broadcast_to(SH))
                nc.vector.tensor_sub(sE[:], sE[:], sD[:])           # -g2
                nc.scalar.activation(sA[:], sE[:], AF.Relu, scale=-1.0)  # gam
                nc.scalar.activation(sD[:], sA[:], AF.Ln)
                nc.scalar.activation(sD[:], sD[:], AF.Exp, scale=0.5)    # g
                nc.scalar.activation(sD[:], sD[:], AF.Identity, bias=1.0)
                nc.vector.reciprocal_approx_fast(sE[:], sD[:])                  # rc
                nc.vector.scalar_tensor_tensor(
                    sB[:], sB[:], 0.7, sE[:], ALU.mult, ALU.mult)   # u_x
                nc.vector.scalar_tensor_tensor(
                    sC[:], sC[:], 0.7, sE[:], ALU.mult, ALU.mult)   # u_v
                nc.scalar.activation(sD[:], sE[:], AF.Square)       # rc^2
                nc.vector.scalar_tensor_tensor(
                    sD[:], sD[:], 0.49, sA[:], ALU.mult, ALU.mult)  # wn2
                nc.vector.scalar_tensor_tensor(
                    sA[:], sB[:], -2.0, sD[:], ALU.mult, ALU.add)
                nc.vector.tensor_add(sA[:], sA[:],
                                     sxx[:].unsqueeze(2)
                                     .broadcast_to(SH))             # zwn2
                nc.vector.reciprocal_approx_fast(sE[:], sA[:])                  # 1/zwn2
                nc.scalar.activation(sA[:], sD[:], AF.Identity,
                                     scale=-1.0, bias=1.0)          # 1-wn2
                nc.vector.tensor_mul(sA[:], sA[:], sE[:])           # a
                nc.scalar.copy(RA[:, :, 1, :], sA[:])
                nc.scalar.activation(sD[:], sA[:], AF.Identity, bias=1.0)  # 1+a

                nc.vector.tensor_mul(sE[:], sD[:],
                                     nx[:].unsqueeze(2)
                                     .broadcast_to(SH))
                nc.vector.tensor_mul(sE[:], sE[:], sC[:])           # hv2
                nc.vector.tensor_mul(sC[:], sD[:], sB[:])           # (1+a)u_x
                nc.vector.tensor_mul(sF[:], sA[:],
                                     sxx[:].unsqueeze(2)
                                     .broadcast_to(SH))
                nc.vector.tensor_sub(sF[:], sC[:], sF[:])           # hr

                # ---- atan2 half-angle; E'-based rad into RA[:, :, 0, :] ----
                nc.vector.scalar_tensor_tensor(
                    sC[:], sF[:], -1.0, sF[:], ALU.mult, ALU.max)   # |hr|
                nc.scalar.activation(sD[:], sF[:], AF.Square)
                nc.scalar.activation(sB[:], sE[:], AF.Square)
                nc.vector.tensor_add(sD[:], sD[:], sB[:])
                nc.scalar.activation(sD[:], sD[:], AF.Ln)
                nc.scalar.activation(sD[:], sD[:], AF.Exp, scale=0.5)  # norm
                nc.vector.tensor_add(sD[:], sD[:], sC[:])           # den
                nc.vector.reciprocal_approx_fast(sC[:], sD[:])
                nc.vector.tensor_mul(sD[:], sE[:], sC[:])           # qq
                nc.scalar.activation(sC[:], sD[:], AF.Square)       # q2
                nc.vector.tensor_scalar(
                    sB[:], sC[:], -2.0 * AT_C3, -2.0 * AT_C2, ALU.mult, ALU.add)
                nc.vector.tensor_mul(sB[:], sB[:], sC[:])
                nc.vector.scalar_tensor_tensor(
                    sB[:], sB[:], -2.0 * AT_C1, sC[:], ALU.add, ALU.mult)
                nc.vector.scalar_tensor_tensor(
                    sB[:], sB[:], -2.0 * AT_C0, sD[:], ALU.add, ALU.mult)
                # sB = E' = -2 atan(qq)

                nc.vector.tensor_scalar(mx8[:], sF[:], 0.0, None, ALU.is_lt)
                nc.vector.tensor_scalar(my8[:], sE[:], 0.0, None, ALU.is_gt)
                nc.vector.tensor_scalar(sF[:], sB[:], -1.0, PI, ALU.mult, ALU.add)
                nc.vector.tensor_scalar(sE[:], sB[:], TWO_PI, None, ALU.add)
                nc.vector.select(sD[:], my8[:], sE[:], sB[:])
                nc.vector.select(RA[:, :, 0, :], mx8[:], sF[:], sD[:])

                # ---- weights + accumulations ----
                nc.scalar.activation(sA[:], wraw, AF.Exp)
                nc.scalar.activation(sA[:], sA[:], AF.Ln, bias=1.0)  # softplus
                nc.vector.tensor_reduce(sumw[:], sA[:], AX.X, ALU.add)
                nc.vector.tensor_mul(WRA[:], RA[:],
                                     sA[:].unsqueeze(2)
                                     .broadcast_to([128, T, 2, K]))
                nc.vector.tensor_reduce(td[:], WRA[:], AX.X, ALU.add)

                # ---- post ----
                nc.vector.reciprocal_approx_fast(rsum[:], sumw[:])
                nc.vector.tensor_mul(tt[:], td[:, :, 0], rsum[:])
                nc.vector.tensor_mul(dtx[:], td[:, :, 1], rsum[:])
                ldj_t = op_.tile(S1, F32, name="ldj_t")
                nc.scalar.activation(ldj_t[:], dtx[:], AF.Ln)

                nc.vector.tensor_scalar(args[:, :, 0], tt[:], -PI, None, ALU.add)
                nc.vector.tensor_scalar(mc[:], tt[:], PI / 2, None, ALU.is_ge)
                nc.vector.tensor_scalar(tsh[:], tt[:], PI / 2, None, ALU.add)
                nc.vector.scalar_tensor_tensor(
                    args[:, :, 1], mc[:], -TWO_PI, tsh[:], ALU.mult, ALU.add)

                nc.vector.tensor_mul(u2[:], args[:], args[:])
                nc.vector.tensor_scalar(
                    hpoly[:], u2[:], SIN_C[5], SIN_C[4], ALU.mult, ALU.add)
                for cf in (SIN_C[3], SIN_C[2], SIN_C[1]):
                    nc.vector.tensor_mul(hpoly[:], hpoly[:], u2[:])
                    nc.vector.tensor_scalar(hpoly[:], hpoly[:], cf, None,
                                            ALU.add)
                nc.vector.tensor_mul(hpoly[:], hpoly[:], u2[:])
                nc.vector.scalar_tensor_tensor(
                    hpoly[:], hpoly[:], SIN_C[0], args[:], ALU.add, ALU.mult)
                # hpoly[...,0] = -sin t ; hpoly[...,1] = cos t

                nc.vector.tensor_mul(r_t[:], x_v,
                                     inv_nx[:].unsqueeze(2)
                                     .broadcast_to(S3))              # x/|x|
                nc.vector.tensor_mul(tx_t[:], r_t[:],
                                     hpoly[:, :, 1:2].broadcast_to(S3))
                nc.vector.tensor_mul(s3a[:], v_t[:],
                                     hpoly[:, :, 0:1].broadcast_to(S3))
                nc.vector.tensor_add(tx_t[:], tx_t[:], s3a[:])
                nc.vector.tensor_scalar(tx_t[:], tx_t[:], -1.0, None, ALU.mult)
                # tx = -(x/|x|)cos + v sin

                nc.gpsimd.tensor_copy(txd[:, :, 0:3], tx_t[:])
                nc.gpsimd.tensor_copy(txd[:, :, 3:6], tx_t[:])
                nc.vector.tensor_mul(s3a[:], txd[:, :, 1:4], ydup[:, :, 2:5])
                nc.vector.tensor_mul(s3b[:], txd[:, :, 2:5], ydup[:, :, 1:4])
                nc.vector.tensor_sub(tz_t[:], s3a[:], s3b[:])
                nc.vector.tensor_mul(s3a[:], tz_t[:], tz_t[:])
                nc.vector.tensor_reduce(tzn[:], s3a[:], AX.X, ALU.add)
                nc.scalar.activation(tzn[:], tzn[:], AF.Ln)
                nc.scalar.activation(tzn[:], tzn[:], AF.Exp, scale=-0.5)
                nc.vector.tensor_mul(tz_t[:], tz_t[:],
                                     tzn[:].unsqueeze(2)
                                     .broadcast_to(S3))

                trot_t = op_.tile([128, T, 9], F32, name="trot_t")
                nc.vector.tensor_copy(trot_t[:, :, 0:9:3], tx_t[:])
                nc.gpsimd.tensor_copy(trot_t[:, :, 1:9:3], y_v)
                nc.scalar.copy(trot_t[:, :, 2:9:3], tz_t[:])

                tvec_t = op_.tile([128, T, 6], F32, name="tvec_t")
                nc.gpsimd.tensor_copy(tvec_t[:, :, 0:3], vec_t[:, :, 0:3])
                nc.vector.tensor_add(tvec_t[:, :, 3:6], vec_t[:, :, 3:6], realb)

                nc.sync.dma_start(
                    trot_o[base:base + ST_ROWS, :]
                    .rearrange("(p t) c -> p t c", t=T), trot_t[:])
                nc.sync.dma_start(
                    tvec_o[base:base + ST_ROWS, :]
                    .rearrange("(p t) c -> p t c", t=T), tvec_t[:])
                nc.sync.dma_start(
                    ldj_o[base:base + ST_ROWS]
                    .rearrange("(p t) -> p t", t=T), ldj_t[:])

    nc.finalize()
    return nc


def _get_nc():
    if "nc" not in _BUILD_CACHE:
        _BUILD_CACHE["nc"] = build_bass()
    return _BUILD_CACHE["nc"]


def _prep_host(W1, b1, W2, b2):
    W1p = np.concatenate([W1[3:259], W1[0:3], W1[259:262]], 0).astype(np.float32)
    idx = list(range(64))
    for c in range(3):
        for k in range(64):
            idx.append(64 + k * 3 + c)
    idx += [259, 260, 261]
    W2p = np.ascontiguousarray(W2[:, idx]).astype(np.float32)
    b2p = np.ascontiguousarray(b2[idx]).astype(np.float32)
    return W1p, np.asarray(b1, np.float32), W2p, b2p


def kernel(rotation, vector, feature, permute, W1, b1, W2, b2, trace=False):
    rotation = np.asarray(rotation)
    vector = np.asarray(vector)
    feature = np.asarray(feature)
    permute = np.asarray(permute)
    assert list(permute) == [0, 1, 2], \
        f"kernel hardcodes permute=(0,1,2), got {permute}"
    N = rotation.shape[0]
    assert N == N_TOTAL

    W1p, b1p, W2p, b2p = _prep_host(
        np.asarray(W1), np.asarray(b1), np.asarray(W2), np.asarray(b2))

    rot_flat = np.ascontiguousarray(rotation.reshape(N, 9), np.float32)
    vec_f = np.ascontiguousarray(vector, np.float32)
    feat_f = np.ascontiguousarray(feature, np.float32)

    nc = _get_nc()
    in_maps = []
    for c in range(NCORES):
        s = slice(c * R, (c + 1) * R)
        in_maps.append(dict(
            rot=rot_flat[s], vec=vec_f[s], feat=feat_f[s],
            W1p=W1p, b1d=b1p, W2p=W2p, b2d=b2p))
    res = run_bass_kernel_spmd(nc, in_maps, core_ids=list(range(NCORES)),
                               trace=trace)
    trot = np.concatenate([r["trot_o"] for r in res.results], 0).reshape(N, 3, 3)
    tvec = np.concatenate([r["tvec_o"] for r in res.results], 0)
    ldj = np.concatenate([r["ldj_o"] for r in res.results], 0)
    if trace:
        return (trot, tvec, ldj), res
    return trot, tvec, ldj
